# revision 55
# baseline (speedup 1.0000x reference)
"""LogSparseAttention Trainium2 kernel.

B,L,H,E = 2,2048,8,64 ; S,D = 2048,64 ; fp32 in/out.
Shard B*H = 16 (b,h) pairs across 8 cores, 2 pairs/core.

Mask structure (reference, rows i >= 22): attend j = i - d for
d in {0..12, 14, 18, 26, 42, 74, 138, 266, 522, 1034}; rows i < 22 are
full causal.  Per 128-key chunk c (j0 = 128c):
  band : scores^T tile K[j]*Q[i], i in [j0, j0+394) -> PSUM band tile
         cols [118, 512) (f32), covers d <= 266
  far  : i in {j0+522+t, j0+1034+t} -> separate PSUM tile [128, 256]
         in BF16, one 256-wide matmul via a strided moving AP
The two far diagonals are never exp'd as stripes: DVE extracts the raw
diagonal scores with one fused tensor_tensor_reduce per stripe (mask =
I128, runs in 2x mode thanks to the bf16 stripe) into band-tile cols
[116, 118), so ONE Act exp covers [116, 512) = diag cols + band: 396
cols instead of 650.  The band mask is applied either by DVE (bf16
multiply) or by PE (a matmul that accumulates -240 into masked-out
score entries BEFORE exp, making them exp to 0) -- per-chunk choice
balances DVE vs PE.

PV accumulates O in NATURAL orientation: out[q-row, e] via flipped
matmuls (lhsT = pAB window piece, rhs = V chunk) into four PSUM
"qgroup" tiles [128, 4*65] (4 query blocks each; V carries a ones
column so e=64 accumulates the softmax denominator Z).  A flipped PV
matmul costs only 65 PE columns, and a qgroup drain is a [128, 260]
copy -- half the column traffic of the O^T layout.  Far contributions
use Vscaled = va * exp_diag (tensor_scalar on the idle Pool engine)
with a shifted-identity lhsT.  The HOST does O = (O[:, :64]/Z).

Q^T / K^T / V(+ones) are pre-laid-out on the HOST so every device DMA
is a contiguous natural load.  SP/HWDGE carries K/Q (small heads
first); consts and V ride the parallel Pool/SWDGE channel.  Drain
copies are emitted >= 2 steps after their qgroup finalizes so the
in-order DVE queue never head-of-line-stalls the mask -> PV chain.
"""

import math

import ml_dtypes
import numpy as np

B, L, H, E = 2, 2048, 8, 64
S, D = 2048, 64
NC_CORES = 8
PAIRS_PER_CORE = 2
CH = L // 128  # 16 chunks
NQG = 4        # qgroup tiles, 4 query blocks each
SCALE = 1.0 / math.sqrt(E)

WBAND = 394                   # band window width: 128 + 266 (d<=266 incl)
FAR0, FAR1 = 522, 1034        # far diagonals (1034 - 522 = 512 -> strided AP)
BOFF = 512 - WBAND            # band at cols [118, 512) of the band tile
QTW = 3488                    # padded Q^T width >= 128*15 + 522 + 1024
QHEAD = 2058                  # q cols for chunks 0-4 incl far rhs
MADD_BIG = -240.0             # additive mask: SCALE*-240 = -30 -> exp ~= 0

# consts tile layout (bf16, [128, CW]):
M0OFF, MGOFF = 0, WBAND                   # multiplicative band masks
IDPOFF = 2 * WBAND                       # identity pair [I | I] (far masks)
CW = 2 * WBAND + 256


# ---------------------------------------------------------------- host masks
def _full_mask() -> np.ndarray:
    """Replica of the reference log-sparse mask [L, S] (0/1 float32)."""
    log_l = math.ceil(math.log2(L))
    m = np.zeros((L, S), dtype=np.float32)
    for index in range(L):
        row = np.zeros(S, dtype=np.float32)
        if (S // L) * 2 * log_l > index:
            row[: index + 1] = 1.0
        else:
            idx = index
            while idx >= 0:
                if idx - log_l + 1 < 0:
                    row[:idx] = 1.0
                    break
                row[idx - log_l + 1 : idx + 1] = 1.0
                for i in range(log_l):
                    new_index = idx - log_l + 1 - 2**i
                    if idx - new_index <= L and new_index >= 0:
                        row[new_index] = 1.0
                idx -= L
        m[index] = row
    return m


_DSET = frozenset(list(range(0, 13)) + [14, 18, 26, 42, 74, 138, 266])


def _band_masks():
    """Two [128, WBAND] 0/1 band masks (chunk 0 | generic), in scores^T
    orientation (row p = key offset, col f = query offset).  Verifies
    that band + the two always-on far diagonals tile the reference mask
    exactly (every nonzero covered exactly once, nothing extra)."""
    mf = _full_mask()
    scatter = np.zeros_like(mf)
    for c in range(CH):
        j0 = 128 * c
        for p in range(128):
            j = j0 + p
            for f in range(WBAND):
                i = j0 + f
                if i >= L:
                    continue
                scatter[i, j] += mf[i, j]  # band cell, read iff mask set
            for dd in (FAR0, FAR1):
                i = j + dd
                if i < L:
                    scatter[i, j] += mf[i, j]
    if not np.array_equal(scatter, mf):
        raise AssertionError("band+far windows do not tile reference mask")
    for dd in (FAR0, FAR1):
        i = np.arange(dd, L)
        assert (mf[i, i - dd] == 1.0).all(), f"far diag {dd} not always attended"
    per_c = []
    for c in (0, 1):
        m = np.zeros((128, WBAND), dtype=np.float32)
        j0 = 128 * c
        for p in range(128):
            for f in range(WBAND):
                i, j = j0 + f, j0 + p
                if i >= L:
                    m[p, f] = 1.0 if (f - p) in _DSET else 0.0
                else:
                    m[p, f] = mf[i, j]
        per_c.append(m)
    # chunks 1..15 all share the generic pattern (checked in earlier
    # kernel generations); chunk c>=2 band rows beyond L were filled
    # with the generic diagonal pattern so the prefix property holds
    mfull = _full_mask()
    for c in range(2, CH):
        j0 = 128 * c
        bw = min(WBAND, L - j0)
        for p in range(128):
            for f in range(bw):
                assert mfull[j0 + f, j0 + p] == per_c[1][p, f]
    return per_c


def _consts_tile():
    m0, mg = _band_masks()
    identp = np.tile(np.eye(128, dtype=np.float32), (1, 2))
    out = np.concatenate([m0, mg, identp], axis=1)
    assert out.shape == (128, CW)
    return out.astype(ml_dtypes.bfloat16)


_CONSTS_NP = _consts_tile()


# ---------------------------------------------------------------- PV pieces
def _band_width(c):
    return min(WBAND, L - 128 * c)


def _nfar(c):
    if 128 * c + FAR1 < L:
        return 2
    if 128 * c + FAR0 < L:
        return 1
    return 0


def _band_pieces(c):
    """Flipped band PV pieces for chunk c: (qblock, rows, lcol).
    out rows [0, rows) of query block qblock <- pAB band cols
    [lcol, lcol+rows)."""
    bw = _band_width(c)
    out = []
    for sub in range(4):
        qb = c + sub
        if 128 * qb >= L:
            break
        w = min(128, bw - 128 * sub)
        if w <= 0:
            break
        out.append((qb, w, 128 * sub))
    return out


def _far_pieces(c):
    """Flipped far PV pieces: (qblock, half, wslot).  lhsT is a
    128-col slice of the zero-padded pfx tile: half 0 covers out rows
    10..127 of qblock (window cols shifted +10), half 1 covers out
    rows 0..9 of the next block (window cols 118..127 at the slice
    head, zeros beyond).  Full-block matmuls; padding rows accumulate
    zeros.  Both far offsets are ==10 mod 128."""
    out = []
    for wi, dd in enumerate((FAR0, FAR1)):
        dst0 = 128 * c + dd
        if dst0 >= L:
            continue
        b0 = dst0 // 128
        out.append((b0, 0, wi))
        if L - dst0 > 118 and 128 * (b0 + 1) < L:
            out.append((b0 + 1, 1, wi))
    return out


# ---------------------------------------------------------------- bass build
_CACHE = {}


def _build_nc():
    import concourse.bacc as bacc
    import concourse.bass as bass
    import concourse.mybir as mybir
    import concourse.tile as tile

    f32 = mybir.dt.float32
    bf16 = mybir.dt.bfloat16
    AF = mybir.ActivationFunctionType
    ALU = mybir.AluOpType

    nc = bacc.Bacc()
    q_d = nc.dram_tensor("q", [PAIRS_PER_CORE, E, QTW], bf16, kind="ExternalInput")
    k_d = nc.dram_tensor("k", [PAIRS_PER_CORE, E, S], bf16, kind="ExternalInput")
    v_d = nc.dram_tensor(
        "v", [PAIRS_PER_CORE, 128, CH * 65], bf16, kind="ExternalInput"
    )
    m_d = nc.dram_tensor("consts", [128, CW], bf16, kind="ExternalInput")
    o_d = nc.dram_tensor(
        "out", [PAIRS_PER_CORE, 128, CH * 65], f32, kind="ExternalOutput"
    )

    with tile.TileContext(nc) as tc:
        with (
            tc.tile_pool(name="const", bufs=1) as constp,
            tc.tile_pool(name="io", bufs=2) as iop,
            tc.tile_pool(name="sc", bufs=8) as scp,
            tc.tile_pool(name="vs", bufs=4) as vsp,
            tc.tile_pool(name="ps", bufs=2, space=bass.MemorySpace.PSUM) as psp,
            tc.tile_pool(name="ot", bufs=1, space=bass.MemorySpace.PSUM) as otp,
        ):
            zc = constp.tile([1, 128], bf16)
            nc.gpsimd.memset(zc[:], 0.0)
            zr = constp.tile([1, 260], bf16)
            nc.gpsimd.memset(zr[:], 0.0)
            # zero-padded far staging tiles: the DVE far-mask writes only
            # the [10, 138) windows, so the padding persists across
            # reuses; memset BEFORE the Pool-channel DMA issues (their
            # descriptor generation occupies Pool for ~1us each and the
            # first far-masks must not wait on these)
            pfxs = [scp.tile([128, 512], bf16, name=f"pfx{j}") for j in range(3)]
            for t in pfxs:
                nc.gpsimd.memset(t[:], 0.0)

            # --- input DMAs.  SP/HWDGE channel: Q head first (longest
            # transfer; QK(0) needs it and the tiny k head -- both sems
            # land ~3.6us), then staged K, then the tails.  Pool/SWDGE
            # channel (parallel): consts, then V in chunks timed so each
            # PV(c) meets its data.  Pair-1 prefetches behind pair-0.
            qts, kts, vas = [], [], []
            consts = None
            for hh in range(PAIRS_PER_CORE):
                qt = iop.tile([E, QTW], bf16, tag="qt")
                kt = iop.tile([E, S], bf16, tag="kt")
                va = iop.tile([128, CH, 65], bf16, tag="va")
                if hh == 0:
                    nc.sync.dma_start(qt[:, 0:QHEAD], q_d[hh][:, 0:QHEAD])
                    nc.sync.dma_start(kt[:, 0:256], k_d[hh][:, 0:256])
                    nc.sync.dma_start(kt[:, 256:1024], k_d[hh][:, 256:1024])
                    nc.sync.dma_start(qt[:, QHEAD:QTW], q_d[hh][:, QHEAD:QTW])
                    nc.sync.dma_start(kt[:, 1024:S], k_d[hh][:, 1024:S])
                    consts = constp.tile([128, CW], bf16)
                    nc.gpsimd.dma_start(consts[:], m_d[:])
                    nc.gpsimd.dma_start(
                        va[:, 0:1, :],
                        v_d[hh][:, 0:65].rearrange("p (c e) -> p c e", c=1),
                    )
                    nc.gpsimd.dma_start(
                        va[:, 1:6, :],
                        v_d[hh][:, 65:390].rearrange("p (c e) -> p c e", c=5),
                    )
                    nc.gpsimd.dma_start(
                        va[:, 6:CH, :],
                        v_d[hh][:, 390 : CH * 65].rearrange(
                            "p (c e) -> p c e", c=CH - 6
                        ),
                    )
                else:
                    nc.sync.dma_start(kt[:], k_d[hh])
                    nc.sync.dma_start(qt[:], q_d[hh])
                    nc.gpsimd.dma_start(
                        va[:], v_d[hh].rearrange("p (c e) -> p c e", c=CH)
                    )
                qts.append(qt)
                kts.append(kt)
                vas.append(va)

            mask0 = consts[:, M0OFF : M0OFF + WBAND]
            maskg = consts[:, MGOFF : MGOFF + WBAND]
            identp = consts[:, IDPOFF : IDPOFF + 256]

            # O accumulator in natural orientation: four [128, 4*65]
            # qgroup tiles (4 query blocks each).  Tile-granular deps
            # mean a drain copy of one qgroup never stalls PVs into the
            # others.  Shared by both pairs sequentially.
            qgt = [otp.tile([128, 4, 65], f32, name=f"qg{g}") for g in range(NQG)]

            def qg_block(qb):
                return qgt[qb // 4][:, qb % 4, :]

            def zinit(g):
                nc.tensor.matmul(
                    qgt[g].rearrange("p c e -> p (c e)"), zc[:], zr[:],
                    start=True, stop=False, skip_group_check=True,
                )

            # Software-pipelined emission over all (pair, chunk) steps.
            # QK + far extraction are emitted 2 steps AHEAD of the tail
            # (exp/mask/PV) so the in-order PE/DVE queues never stall.
            order0 = list(range(12)) + [15, 14, 13, 12]
            steps = [(0, c) for c in order0] + [(1, c) for c in range(CH)]
            ps_tiles = {}
            # SBUF drain staging, one tile per (pair, qgroup)
            ots_tiles = [
                [iop.tile([128, 260], f32, name=f"ots{j}g{g}") for g in range(NQG)]
                for j in range(2)
            ]

            def drain_copy(hh, g, h0, h1):
                # copy half h of qgroup g (cols [130h0, 130h1))
                nc.vector.tensor_copy(
                    ots_tiles[hh][g][:, 130 * h0 : 130 * h1],
                    qgt[g].rearrange("p c e -> p (c e)")[:, 130 * h0 : 130 * h1],
                )

            def drain_dma(hh, g):
                nc.sync.dma_start(
                    o_d[hh][:, 260 * g : 260 * (g + 1)], ots_tiles[hh][g][:]
                )

            # Drain schedule: qgroup g is final after the last chunk
            # writing its blocks (pair-0 order0: qg0 @3, qg1 @7, qg2 @11,
            # qg3 @15; pair-1: qg0 @19, qg1 @23, qg2 @27, qg3 @31).
            # Copies (two 130-col halves) are emitted >= 1 step later so
            # their deps are satisfied when the DVE queue reaches them.
            DRAIN_COPIES = {  # step -> [(hh, qgroup, half0, half1)]
                5: [(0, 0, 0, 1)], 6: [(0, 0, 1, 2)],
                9: [(0, 1, 0, 1)], 10: [(0, 1, 1, 2)],
                13: [(0, 2, 0, 1)], 14: [(0, 2, 1, 2)],
                16: [(0, 3, 0, 1)], 17: [(0, 3, 1, 2)],
                21: [(1, 0, 0, 1)], 22: [(1, 0, 1, 2)],
                25: [(1, 1, 0, 1)], 26: [(1, 1, 1, 2)],
                29: [(1, 2, 0, 1)], 30: [(1, 2, 1, 2)],
            }
            DRAIN_DMAS = {
                6: [(0, 0)], 10: [(0, 1)], 14: [(0, 2)], 17: [(0, 3)],
                22: [(1, 0)], 26: [(1, 1)],
            }
            POOL_DMAS = {30: [(1, 2)]}
            # pair-1 re-zeros, emitted at the END of the step's tail
            # (after the drain copies they must not overtake); each lands
            # before pair-1 first writes the qgroup (qg1: c0's far522
            # piece @16, qg2: c0's far1034 @16 -> both re-zeroed during
            # pair-0 steps; qg3 @18 before c4's far1034 piece @20...
            # actually c8's far522 @24 and c4's far1034 @20; 18 is safe)
            ZINITS = {7: 0, 11: 1, 14: 2, 18: 3}
            # zinit for qg1/qg2 must precede pair-1 c0's far pieces
            # (step 16) -> schedule them at pair-0 steps 11 and 14.

            def emit_qk(i):
                hh, c = steps[i]
                qt, kt = qts[hh], kts[hh]
                j0 = 128 * c
                ktc = kt[:, j0 : j0 + 128]
                bw = _band_width(c)
                nf = _nfar(c)
                psb = psp.tile([128, 1024], f32, tag="psb")
                nc.tensor.matmul(
                    psb[:, BOFF : BOFF + bw], ktc, qt[:, j0 : j0 + bw],
                    start=True, stop=True,
                )
                if nf == 2:
                    rhs = qt[:, j0 + FAR0 : j0 + FAR0 + 1024].rearrange(
                        "p (two x) -> p two x", two=2
                    )[:, :, 0:128]
                    nc.tensor.matmul(
                        psb[:, 512:768], ktc, rhs, start=True, stop=True
                    )
                elif nf == 1:
                    nc.tensor.matmul(
                        psb[:, 512:640], ktc,
                        qt[:, j0 + FAR0 : j0 + FAR0 + 128],
                        start=True, stop=True,
                    )
                ps_tiles[i] = (psb, nf)

            def emit_tail(i):
                hh, c = steps[i]
                psb, nf = ps_tiles.pop(i)
                va = vas[hh]
                bw = _band_width(c)
                w = bw + 128 * nf
                vac = va[:, c, :]
                pAB = scp.tile([128, WBAND + 256], bf16, tag="p")
                # one fused exp: band cols [118, 118+bw) ++ far stripes
                # (the far matmuls land at [512, 768) == [118+394, ...)
                # so the exp range is contiguous)
                nc.scalar.activation(
                    pAB[:, 0:w], psb[:, BOFF : BOFF + w], AF.Exp, scale=SCALE
                )
                mk = mask0 if c == 0 else maskg
                nc.vector.tensor_mul(
                    pAB[:, 0:bw], pAB[:, 0:bw], mk[:, 0:bw]
                )
                pfx = None
                if nf:
                    # far mask: multiply by the diagonal mask, writing
                    # into the zero-padded pfx windows ([256wi+10,
                    # 256wi+138)); the surrounding zeros make the far PV
                    # lhsT slices base-0 full-block matmuls.
                    pfx = pfxs[i % len(pfxs)]
                    nc.vector.tensor_mul(
                        pfx.rearrange("p (w x) -> p w x", w=2)[:, 0:nf, 10:138],
                        pAB[:, bw : bw + 128 * nf].rearrange(
                            "p (w x) -> p w x", w=nf
                        ),
                        identp[:, 0 : 128 * nf].rearrange(
                            "p (w x) -> p w x", w=nf
                        ),
                    )
                # flipped PV matmuls: out rows = query rows, 65 cols each
                for qb, rows, lcol in _band_pieces(c):
                    nc.tensor.matmul(
                        qg_block(qb)[0:rows, :],
                        pAB[:, lcol : lcol + rows],
                        vac,
                        start=False, stop=False, skip_group_check=True,
                    )
                for qb, half, wi in _far_pieces(c):
                    nc.tensor.matmul(
                        qg_block(qb),
                        pfx[:, 256 * wi + 128 * half : 256 * wi + 128 * half + 128],
                        vac,
                        start=False, stop=False, skip_group_check=True,
                    )
                # drain slices AFTER this step's PVs
                for dh, g, h0, h1 in DRAIN_COPIES.get(i, ()):
                    drain_copy(dh, g, h0, h1)
                for dh, g in DRAIN_DMAS.get(i, ()):
                    drain_dma(dh, g)
                for dh, g in POOL_DMAS.get(i, ()):
                    nc.gpsimd.dma_start(
                        o_d[dh][:, 260 * g : 260 * (g + 1)], ots_tiles[dh][g][:]
                    )
                if i in ZINITS:
                    zinit(ZINITS[i])
                if i == 31:
                    # kernel tail: qg2 drained via the Pool DMA above;
                    # qg3's copy (DVE, after its last mask) gates on
                    # PV(31), then one SP/HWDGE DMA ends the kernel.
                    drain_copy(1, 3, 0, 2)
                    drain_dma(1, 3)

            emit_qk(0)
            emit_qk(1)
            for g in range(NQG):
                zinit(g)
            for i in range(len(steps)):
                if i + 2 < len(steps):
                    emit_qk(i + 2)
                emit_tail(i)

    nc.finalize()
    return nc


def _get_nc():
    if "nc" not in _CACHE:
        _CACHE["nc"] = _build_nc()
    return _CACHE["nc"]


# ---------------------------------------------------------------- entrypoint
def kernel(queries, keys, values, attention_mask=None, trace=False):
    from concourse.bass_utils import run_bass_kernel_spmd

    q = np.asarray(queries, dtype=np.float32)
    k = np.asarray(keys, dtype=np.float32)
    v = np.asarray(values, dtype=np.float32)

    # [B, L, H, E] -> [B*H, E, L] (E-major for the device), pad Q cols
    qp = np.ascontiguousarray(q.transpose(0, 2, 3, 1)).reshape(B * H, E, L)
    qpad = np.zeros((B * H, E, QTW), dtype=np.float32)
    qpad[:, :, :L] = qp
    kp = np.ascontiguousarray(k.transpose(0, 2, 3, 1)).reshape(B * H, E, S)
    # V -> [B*H, 128, CH, 65]: v_pre[pair, p, c, e] = V[pair, 128c+p, e],
    # ones column at e=64 (softmax denominator accumulator)
    vp = np.ascontiguousarray(v.transpose(0, 2, 1, 3)).reshape(B * H, S, D)
    vre = vp.reshape(B * H, CH, 128, D).transpose(0, 2, 1, 3)
    vone = np.ones((B * H, 128, CH, 1), dtype=np.float32)
    vpk = np.concatenate([vre, vone], axis=3).reshape(B * H, 128, CH * 65)
    qb = qpad.astype(ml_dtypes.bfloat16)
    kb = kp.astype(ml_dtypes.bfloat16)
    vb = vpk.astype(ml_dtypes.bfloat16)

    in_maps = []
    for m in range(NC_CORES):
        s0 = PAIRS_PER_CORE * m
        in_maps.append(
            {
                "q": np.ascontiguousarray(qb[s0 : s0 + PAIRS_PER_CORE]),
                "k": np.ascontiguousarray(kb[s0 : s0 + PAIRS_PER_CORE]),
                "v": np.ascontiguousarray(vb[s0 : s0 + PAIRS_PER_CORE]),
                "consts": _CONSTS_NP,
            }
        )

    nc = _get_nc()
    res = run_bass_kernel_spmd(
        nc, in_maps, core_ids=list(range(NC_CORES)), trace=trace
    )
    outs = np.stack([r["out"] for r in res.results])  # [8, 2, 128, CH*65]
    on = outs.reshape(B * H, 128, CH, 65).astype(np.float32)
    o = on[..., 0:64] / on[..., 64:65]                 # softmax normalize
    # [BH, p, c, e] -> q = 128c+p -> [B, L, H, D]
    o = o.transpose(0, 2, 1, 3).reshape(B, H, L, D).transpose(0, 2, 1, 3)
    if trace:
        kernel.last_exec_time_ns = res.exec_time_ns
        kernel.last_results = res
    return np.ascontiguousarray(o.astype(np.float32))


# revision 56
# speedup vs baseline: 1.0109x; 1.0109x over previous
"""LogSparseAttention Trainium2 kernel.

B,L,H,E = 2,2048,8,64 ; S,D = 2048,64 ; fp32 in/out.
Shard B*H = 16 (b,h) pairs across 8 cores, 2 pairs/core.

Mask structure (reference, rows i >= 22): attend j = i - d for
d in {0..12, 14, 18, 26, 42, 74, 138, 266, 522, 1034}; rows i < 22 are
full causal.  Per 128-key chunk c (j0 = 128c):
  band : scores^T tile K[j]*Q[i], i in [j0, j0+394) -> PSUM band tile
         cols [118, 512) (f32), covers d <= 266
  far  : i in {j0+522+t, j0+1034+t} -> separate PSUM tile [128, 256]
         in BF16, one 256-wide matmul via a strided moving AP
The two far diagonals are never exp'd as stripes: DVE extracts the raw
diagonal scores with one fused tensor_tensor_reduce per stripe (mask =
I128, runs in 2x mode thanks to the bf16 stripe) into band-tile cols
[116, 118), so ONE Act exp covers [116, 512) = diag cols + band: 396
cols instead of 650.  The band mask is applied either by DVE (bf16
multiply) or by PE (a matmul that accumulates -240 into masked-out
score entries BEFORE exp, making them exp to 0) -- per-chunk choice
balances DVE vs PE.

PV accumulates O in NATURAL orientation: out[q-row, e] via flipped
matmuls (lhsT = pAB window piece, rhs = V chunk) into four PSUM
"qgroup" tiles [128, 4*65] (4 query blocks each; V carries a ones
column so e=64 accumulates the softmax denominator Z).  A flipped PV
matmul costs only 65 PE columns, and a qgroup drain is a [128, 260]
copy -- half the column traffic of the O^T layout.  Far contributions
use Vscaled = va * exp_diag (tensor_scalar on the idle Pool engine)
with a shifted-identity lhsT.  The HOST does O = (O[:, :64]/Z).

Q^T / K^T / V(+ones) are pre-laid-out on the HOST so every device DMA
is a contiguous natural load.  SP/HWDGE carries K/Q (small heads
first); consts and V ride the parallel Pool/SWDGE channel.  Drain
copies are emitted >= 2 steps after their qgroup finalizes so the
in-order DVE queue never head-of-line-stalls the mask -> PV chain.
"""

import math

import ml_dtypes
import numpy as np

B, L, H, E = 2, 2048, 8, 64
S, D = 2048, 64
NC_CORES = 8
PAIRS_PER_CORE = 2
CH = L // 128  # 16 chunks
NQG = 4        # qgroup tiles, 4 query blocks each
SCALE = 1.0 / math.sqrt(E)

WBAND = 394                   # band window width: 128 + 266 (d<=266 incl)
FAR0, FAR1 = 522, 1034        # far diagonals (1034 - 522 = 512 -> strided AP)
BOFF = 512 - WBAND            # band at cols [118, 512) of the band tile
QTW = 3488                    # padded Q^T width >= 128*15 + 522 + 1024
QHEAD = 2058                  # q cols for chunks 0-4 incl far rhs
MADD_BIG = -240.0             # additive mask: SCALE*-240 = -30 -> exp ~= 0

# consts tile layout (bf16, [128, CW]):
M0OFF, MGOFF = 0, WBAND                   # multiplicative band masks
IDPOFF = 2 * WBAND                       # identity pair [I | I] (far masks)
CW = 2 * WBAND + 256


# ---------------------------------------------------------------- host masks
def _full_mask() -> np.ndarray:
    """Replica of the reference log-sparse mask [L, S] (0/1 float32)."""
    log_l = math.ceil(math.log2(L))
    m = np.zeros((L, S), dtype=np.float32)
    for index in range(L):
        row = np.zeros(S, dtype=np.float32)
        if (S // L) * 2 * log_l > index:
            row[: index + 1] = 1.0
        else:
            idx = index
            while idx >= 0:
                if idx - log_l + 1 < 0:
                    row[:idx] = 1.0
                    break
                row[idx - log_l + 1 : idx + 1] = 1.0
                for i in range(log_l):
                    new_index = idx - log_l + 1 - 2**i
                    if idx - new_index <= L and new_index >= 0:
                        row[new_index] = 1.0
                idx -= L
        m[index] = row
    return m


_DSET = frozenset(list(range(0, 13)) + [14, 18, 26, 42, 74, 138, 266])


def _band_masks():
    """Two [128, WBAND] 0/1 band masks (chunk 0 | generic), in scores^T
    orientation (row p = key offset, col f = query offset).  Verifies
    that band + the two always-on far diagonals tile the reference mask
    exactly (every nonzero covered exactly once, nothing extra)."""
    mf = _full_mask()
    scatter = np.zeros_like(mf)
    for c in range(CH):
        j0 = 128 * c
        for p in range(128):
            j = j0 + p
            for f in range(WBAND):
                i = j0 + f
                if i >= L:
                    continue
                scatter[i, j] += mf[i, j]  # band cell, read iff mask set
            for dd in (FAR0, FAR1):
                i = j + dd
                if i < L:
                    scatter[i, j] += mf[i, j]
    if not np.array_equal(scatter, mf):
        raise AssertionError("band+far windows do not tile reference mask")
    for dd in (FAR0, FAR1):
        i = np.arange(dd, L)
        assert (mf[i, i - dd] == 1.0).all(), f"far diag {dd} not always attended"
    per_c = []
    for c in (0, 1):
        m = np.zeros((128, WBAND), dtype=np.float32)
        j0 = 128 * c
        for p in range(128):
            for f in range(WBAND):
                i, j = j0 + f, j0 + p
                if i >= L:
                    m[p, f] = 1.0 if (f - p) in _DSET else 0.0
                else:
                    m[p, f] = mf[i, j]
        per_c.append(m)
    # chunks 1..15 all share the generic pattern (checked in earlier
    # kernel generations); chunk c>=2 band rows beyond L were filled
    # with the generic diagonal pattern so the prefix property holds
    mfull = _full_mask()
    for c in range(2, CH):
        j0 = 128 * c
        bw = min(WBAND, L - j0)
        for p in range(128):
            for f in range(bw):
                assert mfull[j0 + f, j0 + p] == per_c[1][p, f]
    return per_c


def _consts_tile():
    m0, mg = _band_masks()
    identp = np.tile(np.eye(128, dtype=np.float32), (1, 2))
    out = np.concatenate([m0, mg, identp], axis=1)
    assert out.shape == (128, CW)
    return out.astype(ml_dtypes.bfloat16)


_CONSTS_NP = _consts_tile()


# ---------------------------------------------------------------- PV pieces
def _band_width(c):
    return min(WBAND, L - 128 * c)


def _nfar(c):
    if 128 * c + FAR1 < L:
        return 2
    if 128 * c + FAR0 < L:
        return 1
    return 0


def _band_pieces(c):
    """Flipped band PV pieces for chunk c: (qblock, rows, lcol).
    out rows [0, rows) of query block qblock <- pAB band cols
    [lcol, lcol+rows)."""
    bw = _band_width(c)
    out = []
    for sub in range(4):
        qb = c + sub
        if 128 * qb >= L:
            break
        w = min(128, bw - 128 * sub)
        if w <= 0:
            break
        out.append((qb, w, 128 * sub))
    return out


def _far_pieces(c):
    """Flipped far PV pieces: (qblock, half, wslot).  lhsT is a
    128-col slice of the zero-padded pfx tile: half 0 covers out rows
    10..127 of qblock (window cols shifted +10), half 1 covers out
    rows 0..9 of the next block (window cols 118..127 at the slice
    head, zeros beyond).  Full-block matmuls; padding rows accumulate
    zeros.  Both far offsets are ==10 mod 128."""
    out = []
    for wi, dd in enumerate((FAR0, FAR1)):
        dst0 = 128 * c + dd
        if dst0 >= L:
            continue
        b0 = dst0 // 128
        out.append((b0, 0, wi))
        if L - dst0 > 118 and 128 * (b0 + 1) < L:
            out.append((b0 + 1, 1, wi))
    return out


# ---------------------------------------------------------------- bass build
_CACHE = {}


def _build_nc():
    import concourse.bacc as bacc
    import concourse.bass as bass
    import concourse.mybir as mybir
    import concourse.tile as tile

    f32 = mybir.dt.float32
    bf16 = mybir.dt.bfloat16
    AF = mybir.ActivationFunctionType
    ALU = mybir.AluOpType

    nc = bacc.Bacc()
    q_d = nc.dram_tensor("q", [PAIRS_PER_CORE, E, QTW], bf16, kind="ExternalInput")
    k_d = nc.dram_tensor("k", [PAIRS_PER_CORE, E, S], bf16, kind="ExternalInput")
    v_d = nc.dram_tensor(
        "v", [PAIRS_PER_CORE, 128, CH * 65], bf16, kind="ExternalInput"
    )
    m_d = nc.dram_tensor("consts", [128, CW], bf16, kind="ExternalInput")
    o_d = nc.dram_tensor(
        "out", [PAIRS_PER_CORE, 128, CH * 65], f32, kind="ExternalOutput"
    )

    with tile.TileContext(nc) as tc:
        with (
            tc.tile_pool(name="const", bufs=1) as constp,
            tc.tile_pool(name="io", bufs=2) as iop,
            tc.tile_pool(name="sc", bufs=8) as scp,
            tc.tile_pool(name="vs", bufs=4) as vsp,
            tc.tile_pool(name="ps", bufs=2, space=bass.MemorySpace.PSUM) as psp,
            tc.tile_pool(name="ot", bufs=1, space=bass.MemorySpace.PSUM) as otp,
        ):
            zc = constp.tile([1, 128], bf16)
            nc.gpsimd.memset(zc[:], 0.0)
            zr = constp.tile([1, 260], bf16)
            nc.gpsimd.memset(zr[:], 0.0)

            # --- input DMAs.  SP/HWDGE channel: Q head first (longest
            # transfer; QK(0) needs it and the tiny k head -- both sems
            # land ~3.6us), then staged K, then the tails.  Pool/SWDGE
            # channel (parallel): consts, then V in chunks timed so each
            # PV(c) meets its data.  Pair-1 prefetches behind pair-0.
            qts, kts, vas = [], [], []
            consts = None
            for hh in range(PAIRS_PER_CORE):
                qt = iop.tile([E, QTW], bf16, tag="qt")
                kt = iop.tile([E, S], bf16, tag="kt")
                va = iop.tile([128, CH, 65], bf16, tag="va")
                if hh == 0:
                    nc.sync.dma_start(qt[:, 0:QHEAD], q_d[hh][:, 0:QHEAD])
                    nc.sync.dma_start(kt[:, 0:256], k_d[hh][:, 0:256])
                    nc.sync.dma_start(kt[:, 256:1024], k_d[hh][:, 256:1024])
                    nc.sync.dma_start(qt[:, QHEAD:QTW], q_d[hh][:, QHEAD:QTW])
                    nc.sync.dma_start(kt[:, 1024:S], k_d[hh][:, 1024:S])
                    consts = constp.tile([128, CW], bf16)
                    nc.gpsimd.dma_start(consts[:], m_d[:])
                    # zero-padded far staging tiles: the DVE far-mask
                    # writes only the [10, 138) windows, so the padding
                    # persists across reuses; memset right after the
                    # consts DMA issue -- before the V DMAs whose
                    # descriptor generation occupies Pool ~1us each
                    # (the first far-masks must not wait on these)
                    pfxs = [
                        scp.tile([128, 512], bf16, name=f"pfx{j}")
                        for j in range(3)
                    ]
                    for t in pfxs:
                        nc.gpsimd.memset(t[:], 0.0)
                    nc.gpsimd.dma_start(
                        va[:, 0:1, :],
                        v_d[hh][:, 0:65].rearrange("p (c e) -> p c e", c=1),
                    )
                    nc.gpsimd.dma_start(
                        va[:, 1:6, :],
                        v_d[hh][:, 65:390].rearrange("p (c e) -> p c e", c=5),
                    )
                    nc.gpsimd.dma_start(
                        va[:, 6:CH, :],
                        v_d[hh][:, 390 : CH * 65].rearrange(
                            "p (c e) -> p c e", c=CH - 6
                        ),
                    )
                else:
                    nc.sync.dma_start(kt[:], k_d[hh])
                    nc.sync.dma_start(qt[:], q_d[hh])
                    nc.gpsimd.dma_start(
                        va[:], v_d[hh].rearrange("p (c e) -> p c e", c=CH)
                    )
                qts.append(qt)
                kts.append(kt)
                vas.append(va)

            mask0 = consts[:, M0OFF : M0OFF + WBAND]
            maskg = consts[:, MGOFF : MGOFF + WBAND]
            identp = consts[:, IDPOFF : IDPOFF + 256]

            # O accumulator in natural orientation: four [128, 4*65]
            # qgroup tiles (4 query blocks each).  Tile-granular deps
            # mean a drain copy of one qgroup never stalls PVs into the
            # others.  Shared by both pairs sequentially.
            qgt = [otp.tile([128, 4, 65], f32, name=f"qg{g}") for g in range(NQG)]

            def qg_block(qb):
                return qgt[qb // 4][:, qb % 4, :]

            def zinit(g):
                nc.tensor.matmul(
                    qgt[g].rearrange("p c e -> p (c e)"), zc[:], zr[:],
                    start=True, stop=False, skip_group_check=True,
                )

            # Software-pipelined emission over all (pair, chunk) steps.
            # QK + far extraction are emitted 2 steps AHEAD of the tail
            # (exp/mask/PV) so the in-order PE/DVE queues never stall.
            order0 = list(range(12)) + [15, 14, 13, 12]
            steps = [(0, c) for c in order0] + [(1, c) for c in range(CH)]
            ps_tiles = {}
            # SBUF drain staging, one tile per (pair, qgroup)
            ots_tiles = [
                [iop.tile([128, 260], f32, name=f"ots{j}g{g}") for g in range(NQG)]
                for j in range(2)
            ]

            def drain_copy(hh, g, h0, h1):
                # copy half h of qgroup g (cols [130h0, 130h1))
                nc.vector.tensor_copy(
                    ots_tiles[hh][g][:, 130 * h0 : 130 * h1],
                    qgt[g].rearrange("p c e -> p (c e)")[:, 130 * h0 : 130 * h1],
                )

            def drain_dma(hh, g):
                nc.sync.dma_start(
                    o_d[hh][:, 260 * g : 260 * (g + 1)], ots_tiles[hh][g][:]
                )

            # Drain schedule: qgroup g is final after the last chunk
            # writing its blocks (pair-0 order0: qg0 @3, qg1 @7, qg2 @11,
            # qg3 @15; pair-1: qg0 @19, qg1 @23, qg2 @27, qg3 @31).
            # Copies (two 130-col halves) are emitted >= 1 step later so
            # their deps are satisfied when the DVE queue reaches them.
            DRAIN_COPIES = {  # step -> [(hh, qgroup, half0, half1)]
                5: [(0, 0, 0, 1)], 6: [(0, 0, 1, 2)],
                9: [(0, 1, 0, 1)], 10: [(0, 1, 1, 2)],
                13: [(0, 2, 0, 1)], 14: [(0, 2, 1, 2)],
                16: [(0, 3, 0, 1)], 17: [(0, 3, 1, 2)],
                21: [(1, 0, 0, 1)], 22: [(1, 0, 1, 2)],
                25: [(1, 1, 0, 1)], 26: [(1, 1, 1, 2)],
                29: [(1, 2, 0, 1)], 30: [(1, 2, 1, 2)],
            }
            DRAIN_DMAS = {
                6: [(0, 0)], 10: [(0, 1)], 14: [(0, 2)], 17: [(0, 3)],
                22: [(1, 0)], 26: [(1, 1)],
            }
            POOL_DMAS = {30: [(1, 2)]}
            # pair-1 re-zeros, emitted at the END of the step's tail
            # (after the drain copies they must not overtake); each lands
            # before pair-1 first writes the qgroup (qg1: c0's far522
            # piece @16, qg2: c0's far1034 @16 -> both re-zeroed during
            # pair-0 steps; qg3 @18 before c4's far1034 piece @20...
            # actually c8's far522 @24 and c4's far1034 @20; 18 is safe)
            ZINITS = {7: 0, 11: 1, 14: 2, 18: 3}
            # zinit for qg1/qg2 must precede pair-1 c0's far pieces
            # (step 16) -> schedule them at pair-0 steps 11 and 14.

            def emit_qk(i):
                hh, c = steps[i]
                qt, kt = qts[hh], kts[hh]
                j0 = 128 * c
                ktc = kt[:, j0 : j0 + 128]
                bw = _band_width(c)
                nf = _nfar(c)
                psb = psp.tile([128, 1024], f32, tag="psb")
                nc.tensor.matmul(
                    psb[:, BOFF : BOFF + bw], ktc, qt[:, j0 : j0 + bw],
                    start=True, stop=True,
                )
                if nf == 2:
                    rhs = qt[:, j0 + FAR0 : j0 + FAR0 + 1024].rearrange(
                        "p (two x) -> p two x", two=2
                    )[:, :, 0:128]
                    nc.tensor.matmul(
                        psb[:, 512:768], ktc, rhs, start=True, stop=True
                    )
                elif nf == 1:
                    nc.tensor.matmul(
                        psb[:, 512:640], ktc,
                        qt[:, j0 + FAR0 : j0 + FAR0 + 128],
                        start=True, stop=True,
                    )
                ps_tiles[i] = (psb, nf)

            def emit_tail(i):
                hh, c = steps[i]
                psb, nf = ps_tiles.pop(i)
                va = vas[hh]
                bw = _band_width(c)
                w = bw + 128 * nf
                vac = va[:, c, :]
                pAB = scp.tile([128, WBAND + 256], bf16, tag="p")
                # one fused exp: band cols [118, 118+bw) ++ far stripes
                # (the far matmuls land at [512, 768) == [118+394, ...)
                # so the exp range is contiguous)
                nc.scalar.activation(
                    pAB[:, 0:w], psb[:, BOFF : BOFF + w], AF.Exp, scale=SCALE
                )
                mk = mask0 if c == 0 else maskg
                nc.vector.tensor_mul(
                    pAB[:, 0:bw], pAB[:, 0:bw], mk[:, 0:bw]
                )
                pfx = None
                if nf:
                    # far mask: multiply by the diagonal mask, writing
                    # into the zero-padded pfx windows ([256wi+10,
                    # 256wi+138)); the surrounding zeros make the far PV
                    # lhsT slices base-0 full-block matmuls.
                    pfx = pfxs[i % len(pfxs)]
                    nc.vector.tensor_mul(
                        pfx.rearrange("p (w x) -> p w x", w=2)[:, 0:nf, 10:138],
                        pAB[:, bw : bw + 128 * nf].rearrange(
                            "p (w x) -> p w x", w=nf
                        ),
                        identp[:, 0 : 128 * nf].rearrange(
                            "p (w x) -> p w x", w=nf
                        ),
                    )
                # flipped PV matmuls: out rows = query rows, 65 cols each
                for qb, rows, lcol in _band_pieces(c):
                    nc.tensor.matmul(
                        qg_block(qb)[0:rows, :],
                        pAB[:, lcol : lcol + rows],
                        vac,
                        start=False, stop=False, skip_group_check=True,
                    )
                for qb, half, wi in _far_pieces(c):
                    nc.tensor.matmul(
                        qg_block(qb),
                        pfx[:, 256 * wi + 128 * half : 256 * wi + 128 * half + 128],
                        vac,
                        start=False, stop=False, skip_group_check=True,
                    )
                # drain slices AFTER this step's PVs
                for dh, g, h0, h1 in DRAIN_COPIES.get(i, ()):
                    drain_copy(dh, g, h0, h1)
                for dh, g in DRAIN_DMAS.get(i, ()):
                    drain_dma(dh, g)
                for dh, g in POOL_DMAS.get(i, ()):
                    nc.gpsimd.dma_start(
                        o_d[dh][:, 260 * g : 260 * (g + 1)], ots_tiles[dh][g][:]
                    )
                if i in ZINITS:
                    zinit(ZINITS[i])
                if i == 31:
                    # kernel tail: qg2 drained via the Pool DMA above;
                    # qg3's copy (DVE, after its last mask) gates on
                    # PV(31), then one SP/HWDGE DMA ends the kernel.
                    drain_copy(1, 3, 0, 2)
                    drain_dma(1, 3)

            emit_qk(0)
            emit_qk(1)
            for g in range(NQG):
                zinit(g)
            for i in range(len(steps)):
                if i + 2 < len(steps):
                    emit_qk(i + 2)
                emit_tail(i)

    nc.finalize()
    return nc


def _get_nc():
    if "nc" not in _CACHE:
        _CACHE["nc"] = _build_nc()
    return _CACHE["nc"]


# ---------------------------------------------------------------- entrypoint
def kernel(queries, keys, values, attention_mask=None, trace=False):
    from concourse.bass_utils import run_bass_kernel_spmd

    q = np.asarray(queries, dtype=np.float32)
    k = np.asarray(keys, dtype=np.float32)
    v = np.asarray(values, dtype=np.float32)

    # [B, L, H, E] -> [B*H, E, L] (E-major for the device), pad Q cols
    qp = np.ascontiguousarray(q.transpose(0, 2, 3, 1)).reshape(B * H, E, L)
    qpad = np.zeros((B * H, E, QTW), dtype=np.float32)
    qpad[:, :, :L] = qp
    kp = np.ascontiguousarray(k.transpose(0, 2, 3, 1)).reshape(B * H, E, S)
    # V -> [B*H, 128, CH, 65]: v_pre[pair, p, c, e] = V[pair, 128c+p, e],
    # ones column at e=64 (softmax denominator accumulator)
    vp = np.ascontiguousarray(v.transpose(0, 2, 1, 3)).reshape(B * H, S, D)
    vre = vp.reshape(B * H, CH, 128, D).transpose(0, 2, 1, 3)
    vone = np.ones((B * H, 128, CH, 1), dtype=np.float32)
    vpk = np.concatenate([vre, vone], axis=3).reshape(B * H, 128, CH * 65)
    qb = qpad.astype(ml_dtypes.bfloat16)
    kb = kp.astype(ml_dtypes.bfloat16)
    vb = vpk.astype(ml_dtypes.bfloat16)

    in_maps = []
    for m in range(NC_CORES):
        s0 = PAIRS_PER_CORE * m
        in_maps.append(
            {
                "q": np.ascontiguousarray(qb[s0 : s0 + PAIRS_PER_CORE]),
                "k": np.ascontiguousarray(kb[s0 : s0 + PAIRS_PER_CORE]),
                "v": np.ascontiguousarray(vb[s0 : s0 + PAIRS_PER_CORE]),
                "consts": _CONSTS_NP,
            }
        )

    nc = _get_nc()
    res = run_bass_kernel_spmd(
        nc, in_maps, core_ids=list(range(NC_CORES)), trace=trace
    )
    outs = np.stack([r["out"] for r in res.results])  # [8, 2, 128, CH*65]
    on = outs.reshape(B * H, 128, CH, 65).astype(np.float32)
    o = on[..., 0:64] / on[..., 64:65]                 # softmax normalize
    # [BH, p, c, e] -> q = 128c+p -> [B, L, H, D]
    o = o.transpose(0, 2, 1, 3).reshape(B, H, L, D).transpose(0, 2, 1, 3)
    if trace:
        kernel.last_exec_time_ns = res.exec_time_ns
        kernel.last_results = res
    return np.ascontiguousarray(o.astype(np.float32))


# revision 57
# speedup vs baseline: 1.0729x; 1.0613x over previous
"""LogSparseAttention Trainium2 kernel.

B,L,H,E = 2,2048,8,64 ; S,D = 2048,64 ; fp32 in/out.
Shard B*H = 16 (b,h) pairs across 8 cores, 2 pairs/core.

Mask structure (reference, rows i >= 22): attend j = i - d for
d in {0..12, 14, 18, 26, 42, 74, 138, 266, 522, 1034}; rows i < 22 are
full causal.  Per 128-key chunk c (j0 = 128c):
  band : scores^T tile K[j]*Q[i], i in [j0, j0+394) -> PSUM band tile
         cols [118, 512) (f32), covers d <= 266
  far  : i in {j0+522+t, j0+1034+t} -> separate PSUM tile [128, 256]
         in BF16, one 256-wide matmul via a strided moving AP
The two far diagonals are never exp'd as stripes: DVE extracts the raw
diagonal scores with one fused tensor_tensor_reduce per stripe (mask =
I128, runs in 2x mode thanks to the bf16 stripe) into band-tile cols
[116, 118), so ONE Act exp covers [116, 512) = diag cols + band: 396
cols instead of 650.  The band mask is applied either by DVE (bf16
multiply) or by PE (a matmul that accumulates -240 into masked-out
score entries BEFORE exp, making them exp to 0) -- per-chunk choice
balances DVE vs PE.

PV accumulates O in NATURAL orientation: out[q-row, e] via flipped
matmuls (lhsT = pAB window piece, rhs = V chunk) into four PSUM
"qgroup" tiles [128, 4*65] (4 query blocks each; V carries a ones
column so e=64 accumulates the softmax denominator Z).  A flipped PV
matmul costs only 65 PE columns, and a qgroup drain is a [128, 260]
copy -- half the column traffic of the O^T layout.  Far contributions
use Vscaled = va * exp_diag (tensor_scalar on the idle Pool engine)
with a shifted-identity lhsT.  The HOST does O = (O[:, :64]/Z).

Q^T / K^T / V(+ones) are pre-laid-out on the HOST so every device DMA
is a contiguous natural load.  SP/HWDGE carries K/Q (small heads
first); consts and V ride the parallel Pool/SWDGE channel.  Drain
copies are emitted >= 2 steps after their qgroup finalizes so the
in-order DVE queue never head-of-line-stalls the mask -> PV chain.
"""

import math

import ml_dtypes
import numpy as np

B, L, H, E = 2, 2048, 8, 64
S, D = 2048, 64
NC_CORES = 8
PAIRS_PER_CORE = 2
CH = L // 128  # 16 chunks
NQG = 4        # qgroup tiles, 4 query blocks each
SCALE = 1.0 / math.sqrt(E)

WBAND = 394                   # band window width: 128 + 266 (d<=266 incl)
FAR0, FAR1 = 522, 1034        # far diagonals (1034 - 522 = 512 -> strided AP)
BOFF = 512 - WBAND            # band at cols [118, 512) of the band tile
QTW = 3488                    # padded Q^T width >= 128*15 + 522 + 1024
QHEAD = 2058                  # q cols for chunks 0-4 incl far rhs
MADD_BIG = -240.0             # additive mask: SCALE*-240 = -30 -> exp ~= 0

# consts tile layout (bf16, [128, CW]):
M0OFF, MGOFF = 0, WBAND                   # multiplicative band masks
IDPOFF = 2 * WBAND                       # identity pair [I | I] (far masks)
CW = 2 * WBAND + 256


# ---------------------------------------------------------------- host masks
def _full_mask() -> np.ndarray:
    """Replica of the reference log-sparse mask [L, S] (0/1 float32)."""
    log_l = math.ceil(math.log2(L))
    m = np.zeros((L, S), dtype=np.float32)
    for index in range(L):
        row = np.zeros(S, dtype=np.float32)
        if (S // L) * 2 * log_l > index:
            row[: index + 1] = 1.0
        else:
            idx = index
            while idx >= 0:
                if idx - log_l + 1 < 0:
                    row[:idx] = 1.0
                    break
                row[idx - log_l + 1 : idx + 1] = 1.0
                for i in range(log_l):
                    new_index = idx - log_l + 1 - 2**i
                    if idx - new_index <= L and new_index >= 0:
                        row[new_index] = 1.0
                idx -= L
        m[index] = row
    return m


_DSET = frozenset(list(range(0, 13)) + [14, 18, 26, 42, 74, 138, 266])


def _band_masks():
    """Two [128, WBAND] 0/1 band masks (chunk 0 | generic), in scores^T
    orientation (row p = key offset, col f = query offset).  Verifies
    that band + the two always-on far diagonals tile the reference mask
    exactly (every nonzero covered exactly once, nothing extra)."""
    mf = _full_mask()
    scatter = np.zeros_like(mf)
    for c in range(CH):
        j0 = 128 * c
        for p in range(128):
            j = j0 + p
            for f in range(WBAND):
                i = j0 + f
                if i >= L:
                    continue
                scatter[i, j] += mf[i, j]  # band cell, read iff mask set
            for dd in (FAR0, FAR1):
                i = j + dd
                if i < L:
                    scatter[i, j] += mf[i, j]
    if not np.array_equal(scatter, mf):
        raise AssertionError("band+far windows do not tile reference mask")
    for dd in (FAR0, FAR1):
        i = np.arange(dd, L)
        assert (mf[i, i - dd] == 1.0).all(), f"far diag {dd} not always attended"
    per_c = []
    for c in (0, 1):
        m = np.zeros((128, WBAND), dtype=np.float32)
        j0 = 128 * c
        for p in range(128):
            for f in range(WBAND):
                i, j = j0 + f, j0 + p
                if i >= L:
                    m[p, f] = 1.0 if (f - p) in _DSET else 0.0
                else:
                    m[p, f] = mf[i, j]
        per_c.append(m)
    # chunks 1..15 all share the generic pattern (checked in earlier
    # kernel generations); chunk c>=2 band rows beyond L were filled
    # with the generic diagonal pattern so the prefix property holds
    mfull = _full_mask()
    for c in range(2, CH):
        j0 = 128 * c
        bw = min(WBAND, L - j0)
        for p in range(128):
            for f in range(bw):
                assert mfull[j0 + f, j0 + p] == per_c[1][p, f]
    return per_c


def _consts_tile():
    m0, mg = _band_masks()
    identp = np.tile(np.eye(128, dtype=np.float32), (1, 2))
    out = np.concatenate([m0, mg, identp], axis=1)
    assert out.shape == (128, CW)
    return out.astype(ml_dtypes.bfloat16)


_CONSTS_NP = _consts_tile()


# ---------------------------------------------------------------- PV pieces
def _band_width(c):
    return min(WBAND, L - 128 * c)


def _nfar(c):
    if 128 * c + FAR1 < L:
        return 2
    if 128 * c + FAR0 < L:
        return 1
    return 0


def _band_pieces(c):
    """Flipped band PV pieces for chunk c: (qblock, rows, lcol).
    out rows [0, rows) of query block qblock <- pAB band cols
    [lcol, lcol+rows)."""
    bw = _band_width(c)
    out = []
    for sub in range(4):
        qb = c + sub
        if 128 * qb >= L:
            break
        w = min(128, bw - 128 * sub)
        if w <= 0:
            break
        out.append((qb, w, 128 * sub))
    return out


def _far_pieces(c):
    """Flipped far PV pieces: (qblock, half, wslot).  lhsT is a
    128-col slice of the zero-padded pfx tile: half 0 covers out rows
    10..127 of qblock (window cols shifted +10), half 1 covers out
    rows 0..9 of the next block (window cols 118..127 at the slice
    head, zeros beyond).  Full-block matmuls; padding rows accumulate
    zeros.  Both far offsets are ==10 mod 128."""
    out = []
    for wi, dd in enumerate((FAR0, FAR1)):
        dst0 = 128 * c + dd
        if dst0 >= L:
            continue
        b0 = dst0 // 128
        out.append((b0, 0, wi))
        if L - dst0 > 118 and 128 * (b0 + 1) < L:
            out.append((b0 + 1, 1, wi))
    return out


# ---------------------------------------------------------------- bass build
_CACHE = {}


def _build_nc():
    import concourse.bacc as bacc
    import concourse.bass as bass
    import concourse.mybir as mybir
    import concourse.tile as tile

    f32 = mybir.dt.float32
    bf16 = mybir.dt.bfloat16
    AF = mybir.ActivationFunctionType
    ALU = mybir.AluOpType

    nc = bacc.Bacc()
    q_d = nc.dram_tensor("q", [PAIRS_PER_CORE, E, QTW], bf16, kind="ExternalInput")
    k_d = nc.dram_tensor("k", [PAIRS_PER_CORE, E, S], bf16, kind="ExternalInput")
    v_d = nc.dram_tensor(
        "v", [PAIRS_PER_CORE, 128, CH * 65], bf16, kind="ExternalInput"
    )
    m_d = nc.dram_tensor("consts", [128, CW], bf16, kind="ExternalInput")
    o_d = nc.dram_tensor(
        "out", [PAIRS_PER_CORE, 128, CH * 65], f32, kind="ExternalOutput"
    )

    with tile.TileContext(nc) as tc:
        with (
            tc.tile_pool(name="const", bufs=1) as constp,
            tc.tile_pool(name="io", bufs=2) as iop,
            tc.tile_pool(name="sc", bufs=8) as scp,
            tc.tile_pool(name="vs", bufs=4) as vsp,
            tc.tile_pool(name="ps", bufs=2, space=bass.MemorySpace.PSUM) as psp,
            tc.tile_pool(name="ot", bufs=1, space=bass.MemorySpace.PSUM) as otp,
        ):
            zc = constp.tile([1, 128], bf16)
            nc.gpsimd.memset(zc[:], 0.0)
            zr = constp.tile([1, 260], bf16)
            nc.gpsimd.memset(zr[:], 0.0)

            # --- input DMAs.  SP/HWDGE channel: Q head first (longest
            # transfer; QK(0) needs it and the tiny k head -- both sems
            # land ~3.6us), then staged K, then the tails.  Pool/SWDGE
            # channel (parallel): consts, then V in chunks timed so each
            # PV(c) meets its data.  Pair-1 prefetches behind pair-0.
            qts, kts, vas = [], [], []
            consts = None
            for hh in range(PAIRS_PER_CORE):
                qt = iop.tile([E, QTW], bf16, tag="qt")
                kt = iop.tile([E, S], bf16, tag="kt")
                va = iop.tile([128, CH, 65], bf16, tag="va")
                if hh == 0:
                    nc.sync.dma_start(qt[:, 0:QHEAD], q_d[hh][:, 0:QHEAD])
                    nc.sync.dma_start(kt[:, 0:256], k_d[hh][:, 0:256])
                    nc.sync.dma_start(kt[:, 256:1024], k_d[hh][:, 256:1024])
                    nc.sync.dma_start(qt[:, QHEAD:QTW], q_d[hh][:, QHEAD:QTW])
                    nc.sync.dma_start(kt[:, 1024:S], k_d[hh][:, 1024:S])
                    consts = constp.tile([128, CW], bf16)
                    nc.gpsimd.dma_start(consts[:], m_d[:])
                    # zero-padded far staging tiles: the DVE far-mask
                    # writes only the [10, 138) windows, so the padding
                    # persists across reuses; memset on DVE, which idles
                    # until the first band mask ~5us in
                    pfxs = [
                        scp.tile([128, 512], bf16, name=f"pfx{j}")
                        for j in range(3)
                    ]
                    for t in pfxs:
                        nc.vector.memset(t[:], 0.0)
                    nc.gpsimd.dma_start(
                        va[:, 0:1, :],
                        v_d[hh][:, 0:65].rearrange("p (c e) -> p c e", c=1),
                    )
                    nc.gpsimd.dma_start(
                        va[:, 1:6, :],
                        v_d[hh][:, 65:390].rearrange("p (c e) -> p c e", c=5),
                    )
                    nc.gpsimd.dma_start(
                        va[:, 6:CH, :],
                        v_d[hh][:, 390 : CH * 65].rearrange(
                            "p (c e) -> p c e", c=CH - 6
                        ),
                    )
                else:
                    nc.sync.dma_start(kt[:], k_d[hh])
                    nc.sync.dma_start(qt[:], q_d[hh])
                    nc.gpsimd.dma_start(
                        va[:], v_d[hh].rearrange("p (c e) -> p c e", c=CH)
                    )
                qts.append(qt)
                kts.append(kt)
                vas.append(va)

            mask0 = consts[:, M0OFF : M0OFF + WBAND]
            maskg = consts[:, MGOFF : MGOFF + WBAND]
            identp = consts[:, IDPOFF : IDPOFF + 256]

            # O accumulator in natural orientation: four [128, 4*65]
            # qgroup tiles (4 query blocks each).  Tile-granular deps
            # mean a drain copy of one qgroup never stalls PVs into the
            # others.  Shared by both pairs sequentially.
            qgt = [otp.tile([128, 4, 65], f32, name=f"qg{g}") for g in range(NQG)]

            def qg_block(qb):
                return qgt[qb // 4][:, qb % 4, :]

            def zinit(g):
                nc.tensor.matmul(
                    qgt[g].rearrange("p c e -> p (c e)"), zc[:], zr[:],
                    start=True, stop=False, skip_group_check=True,
                )

            # Software-pipelined emission over all (pair, chunk) steps.
            # QK + far extraction are emitted 2 steps AHEAD of the tail
            # (exp/mask/PV) so the in-order PE/DVE queues never stall.
            order0 = list(range(12)) + [15, 14, 13, 12]
            steps = [(0, c) for c in order0] + [(1, c) for c in range(CH)]
            ps_tiles = {}
            # SBUF drain staging, one tile per (pair, qgroup)
            ots_tiles = [
                [iop.tile([128, 260], f32, name=f"ots{j}g{g}") for g in range(NQG)]
                for j in range(2)
            ]

            def drain_copy(hh, g, h0, h1):
                # copy half h of qgroup g (cols [130h0, 130h1))
                nc.vector.tensor_copy(
                    ots_tiles[hh][g][:, 130 * h0 : 130 * h1],
                    qgt[g].rearrange("p c e -> p (c e)")[:, 130 * h0 : 130 * h1],
                )

            def drain_dma(hh, g):
                nc.sync.dma_start(
                    o_d[hh][:, 260 * g : 260 * (g + 1)], ots_tiles[hh][g][:]
                )

            # Drain schedule: qgroup g is final after the last chunk
            # writing its blocks (pair-0 order0: qg0 @3, qg1 @7, qg2 @11,
            # qg3 @15; pair-1: qg0 @19, qg1 @23, qg2 @27, qg3 @31).
            # Copies (two 130-col halves) are emitted >= 1 step later so
            # their deps are satisfied when the DVE queue reaches them.
            DRAIN_COPIES = {  # step -> [(hh, qgroup, half0, half1)]
                5: [(0, 0, 0, 1)], 6: [(0, 0, 1, 2)],
                9: [(0, 1, 0, 1)], 10: [(0, 1, 1, 2)],
                13: [(0, 2, 0, 1)], 14: [(0, 2, 1, 2)],
                16: [(0, 3, 0, 1)], 17: [(0, 3, 1, 2)],
                21: [(1, 0, 0, 1)], 22: [(1, 0, 1, 2)],
                25: [(1, 1, 0, 1)], 26: [(1, 1, 1, 2)],
                29: [(1, 2, 0, 1)], 30: [(1, 2, 1, 2)],
            }
            DRAIN_DMAS = {
                6: [(0, 0)], 10: [(0, 1)], 14: [(0, 2)], 17: [(0, 3)],
                22: [(1, 0)], 26: [(1, 1)],
            }
            POOL_DMAS = {30: [(1, 2)]}
            # pair-1 re-zeros, emitted at the END of the step's tail
            # (after the drain copies they must not overtake); each lands
            # before pair-1 first writes the qgroup (qg1: c0's far522
            # piece @16, qg2: c0's far1034 @16 -> both re-zeroed during
            # pair-0 steps; qg3 @18 before c4's far1034 piece @20...
            # actually c8's far522 @24 and c4's far1034 @20; 18 is safe)
            ZINITS = {7: 0, 11: 1, 14: 2, 18: 3}
            # zinit for qg1/qg2 must precede pair-1 c0's far pieces
            # (step 16) -> schedule them at pair-0 steps 11 and 14.

            def emit_qk(i):
                hh, c = steps[i]
                qt, kt = qts[hh], kts[hh]
                j0 = 128 * c
                ktc = kt[:, j0 : j0 + 128]
                bw = _band_width(c)
                nf = _nfar(c)
                psb = psp.tile([128, 1024], f32, tag="psb")
                nc.tensor.matmul(
                    psb[:, BOFF : BOFF + bw], ktc, qt[:, j0 : j0 + bw],
                    start=True, stop=True,
                )
                if nf == 2:
                    rhs = qt[:, j0 + FAR0 : j0 + FAR0 + 1024].rearrange(
                        "p (two x) -> p two x", two=2
                    )[:, :, 0:128]
                    nc.tensor.matmul(
                        psb[:, 512:768], ktc, rhs, start=True, stop=True
                    )
                elif nf == 1:
                    nc.tensor.matmul(
                        psb[:, 512:640], ktc,
                        qt[:, j0 + FAR0 : j0 + FAR0 + 128],
                        start=True, stop=True,
                    )
                ps_tiles[i] = (psb, nf)

            def emit_tail(i):
                hh, c = steps[i]
                psb, nf = ps_tiles.pop(i)
                va = vas[hh]
                bw = _band_width(c)
                w = bw + 128 * nf
                vac = va[:, c, :]
                pAB = scp.tile([128, WBAND + 256], bf16, tag="p")
                # one fused exp: band cols [118, 118+bw) ++ far stripes
                # (the far matmuls land at [512, 768) == [118+394, ...)
                # so the exp range is contiguous)
                nc.scalar.activation(
                    pAB[:, 0:w], psb[:, BOFF : BOFF + w], AF.Exp, scale=SCALE
                )
                mk = mask0 if c == 0 else maskg
                nc.vector.tensor_mul(
                    pAB[:, 0:bw], pAB[:, 0:bw], mk[:, 0:bw]
                )
                pfx = None
                if nf:
                    # far mask: multiply by the diagonal mask, writing
                    # into the zero-padded pfx windows ([256wi+10,
                    # 256wi+138)); the surrounding zeros make the far PV
                    # lhsT slices base-0 full-block matmuls.
                    pfx = pfxs[i % len(pfxs)]
                    nc.vector.tensor_mul(
                        pfx.rearrange("p (w x) -> p w x", w=2)[:, 0:nf, 10:138],
                        pAB[:, bw : bw + 128 * nf].rearrange(
                            "p (w x) -> p w x", w=nf
                        ),
                        identp[:, 0 : 128 * nf].rearrange(
                            "p (w x) -> p w x", w=nf
                        ),
                    )
                # flipped PV matmuls: out rows = query rows, 65 cols each
                for qb, rows, lcol in _band_pieces(c):
                    nc.tensor.matmul(
                        qg_block(qb)[0:rows, :],
                        pAB[:, lcol : lcol + rows],
                        vac,
                        start=False, stop=False, skip_group_check=True,
                    )
                for qb, half, wi in _far_pieces(c):
                    nc.tensor.matmul(
                        qg_block(qb),
                        pfx[:, 256 * wi + 128 * half : 256 * wi + 128 * half + 128],
                        vac,
                        start=False, stop=False, skip_group_check=True,
                    )
                # drain slices AFTER this step's PVs
                for dh, g, h0, h1 in DRAIN_COPIES.get(i, ()):
                    drain_copy(dh, g, h0, h1)
                for dh, g in DRAIN_DMAS.get(i, ()):
                    drain_dma(dh, g)
                for dh, g in POOL_DMAS.get(i, ()):
                    nc.gpsimd.dma_start(
                        o_d[dh][:, 260 * g : 260 * (g + 1)], ots_tiles[dh][g][:]
                    )
                if i in ZINITS:
                    zinit(ZINITS[i])
                if i == 31:
                    # kernel tail: qg2 drained via the Pool DMA above;
                    # qg3's copy (DVE, after its last mask) gates on
                    # PV(31), then one SP/HWDGE DMA ends the kernel.
                    drain_copy(1, 3, 0, 2)
                    drain_dma(1, 3)

            emit_qk(0)
            emit_qk(1)
            for g in range(NQG):
                zinit(g)
            for i in range(len(steps)):
                if i + 2 < len(steps):
                    emit_qk(i + 2)
                emit_tail(i)

    nc.finalize()
    return nc


def _get_nc():
    if "nc" not in _CACHE:
        _CACHE["nc"] = _build_nc()
    return _CACHE["nc"]


# ---------------------------------------------------------------- entrypoint
def kernel(queries, keys, values, attention_mask=None, trace=False):
    from concourse.bass_utils import run_bass_kernel_spmd

    q = np.asarray(queries, dtype=np.float32)
    k = np.asarray(keys, dtype=np.float32)
    v = np.asarray(values, dtype=np.float32)

    # [B, L, H, E] -> [B*H, E, L] (E-major for the device), pad Q cols
    qp = np.ascontiguousarray(q.transpose(0, 2, 3, 1)).reshape(B * H, E, L)
    qpad = np.zeros((B * H, E, QTW), dtype=np.float32)
    qpad[:, :, :L] = qp
    kp = np.ascontiguousarray(k.transpose(0, 2, 3, 1)).reshape(B * H, E, S)
    # V -> [B*H, 128, CH, 65]: v_pre[pair, p, c, e] = V[pair, 128c+p, e],
    # ones column at e=64 (softmax denominator accumulator)
    vp = np.ascontiguousarray(v.transpose(0, 2, 1, 3)).reshape(B * H, S, D)
    vre = vp.reshape(B * H, CH, 128, D).transpose(0, 2, 1, 3)
    vone = np.ones((B * H, 128, CH, 1), dtype=np.float32)
    vpk = np.concatenate([vre, vone], axis=3).reshape(B * H, 128, CH * 65)
    qb = qpad.astype(ml_dtypes.bfloat16)
    kb = kp.astype(ml_dtypes.bfloat16)
    vb = vpk.astype(ml_dtypes.bfloat16)

    in_maps = []
    for m in range(NC_CORES):
        s0 = PAIRS_PER_CORE * m
        in_maps.append(
            {
                "q": np.ascontiguousarray(qb[s0 : s0 + PAIRS_PER_CORE]),
                "k": np.ascontiguousarray(kb[s0 : s0 + PAIRS_PER_CORE]),
                "v": np.ascontiguousarray(vb[s0 : s0 + PAIRS_PER_CORE]),
                "consts": _CONSTS_NP,
            }
        )

    nc = _get_nc()
    res = run_bass_kernel_spmd(
        nc, in_maps, core_ids=list(range(NC_CORES)), trace=trace
    )
    outs = np.stack([r["out"] for r in res.results])  # [8, 2, 128, CH*65]
    on = outs.reshape(B * H, 128, CH, 65).astype(np.float32)
    o = on[..., 0:64] / on[..., 64:65]                 # softmax normalize
    # [BH, p, c, e] -> q = 128c+p -> [B, L, H, D]
    o = o.transpose(0, 2, 1, 3).reshape(B, H, L, D).transpose(0, 2, 1, 3)
    if trace:
        kernel.last_exec_time_ns = res.exec_time_ns
        kernel.last_results = res
    return np.ascontiguousarray(o.astype(np.float32))
